# revision 46
# baseline (speedup 1.0000x reference)
"""MoE routing kernel for Trainium2 (8 NeuronCores).

Reference computation (B=16384, IN=64, HID=128, OUT=1, E=64, NMAP=1000):
    e = c[num]                                  # [B] expert id per sample
    h = relu(x @ W1[e] + b1[e])                 # [B, HID]
    y = sigmoid(h @ W2[e] + b2[e])              # [B, OUT]

Strategy: sort-by-expert dispatch on the host (the routing is pure
integer bookkeeping), dense per-expert matmuls on device. Each core gets
the same static slot structure (SPMD: one graph for all 8 cores); slot
widths are equalized across cores by snake-dealing the per-expert chunks
in descending size order, padding each slot to the max width over cores.

Device layout (per core): slots are paired onto the 128 SBUF partitions
— pair p puts slot 2p's x^T on partitions 0:64 and slot 2p+1's on
64:128. Full-width DMA, and the two K=64 matmuls of a pair run
concurrently in disjoint PE row groups. Slots are also first-fit packed
into "bins" of <=512 y columns: each bin is one PSUM bank, one
block-diagonal mm2 (lhsT = the bin's w2 columns), and one sigmoid.
All tensor data is bf16 (rel-err budget 2e-2); accumulation stays f32.

Per slot j (width Wj <= 512, pair p, bin b):
    mm1:   psum1[HID=128, Wj] = W1_j[64,128].T @ xT[64, Wj]      (PE)
    relu:  hbin_b[:, cj:cj+Wj] = bf16(max(psum1 + b1_j, 0))      (DVE)
Per bin b (M slots, width Wb <= 512):
    mm2:   psum2[M, Wb] = w2_bin[128,M].T @ hbin_b[128, Wb]      (PE)
    sig:   y[0:M, bin] = sigmoid(psum2 + b2_bin[M,1])            (ACT)
Slot j's outputs live in y[row_of_j_in_bin, its columns] (the
off-diagonal rows are garbage the host ignores).
"""

import os
import sys

if "/opt/trn_rl_repo" not in sys.path:
    sys.path.insert(0, "/opt/trn_rl_repo")

import numpy as np

import concourse.bass as bass
import concourse.mybir as mybir
from concourse import tile
from concourse.bass_utils import run_bass_kernel_spmd

N_CORES = 8
IN = 64
HID = 128
E = 64
MAX_W = 512  # moving-operand / PSUM-bank limit

BF16 = mybir.dt.bfloat16
F32 = mybir.dt.float32
NP_BF16 = mybir.dt.np(BF16)


# ---------------------------------------------------------------------------
# This container's walrus build rejects more than one sync wait per
# instruction ("Too many sync wait commands"). Post-pass over the lowered
# BIR: move the extra waits onto single-wait NOPs inserted just before the
# instruction on the same engine (program order makes this equivalent).
# ---------------------------------------------------------------------------
def _split_multi_waits(nc):
    ctr = 0
    for f in nc.m.functions:
        for blk in f.blocks:
            new_list = []
            for ins in blk.instructions:
                si = ins.sync_info
                if si is not None and si.on_wait and len(si.on_wait) > 1:
                    waits = list(si.on_wait)
                    for w in waits[:-1]:
                        ctr += 1
                        new_list.append(
                            mybir.InstNoOp(
                                name=f"waitsplit-{ctr}",
                                engine=ins.engine,
                                bass_nofuse=True,
                                sync_info=mybir.SyncInfo(
                                    on_wait=[w], on_update=[]
                                ),
                            )
                        )
                    si.on_wait = waits[-1:]
                    ins.sync_info = si
                new_list.append(ins)
            blk.instructions = new_list


def _filter_drain_waits(nc, out_dma_names):
    """The kernel-tail drain only needs to gate on the output DMAs'
    completion semaphores — every other wait Tile put on it is
    transitively implied. Fewer waits = fewer single-wait NOPs."""
    keep_ids = set()
    drain = None
    for f in nc.m.functions:
        for blk in f.blocks:
            for ins in blk.instructions:
                if ins.name in out_dma_names and ins.sync_info is not None:
                    for u in ins.sync_info.on_update:
                        keep_ids.add(u.id)
                if isinstance(ins, mybir.InstDrain):
                    si = ins.sync_info
                    if si is not None and len(si.on_wait) > 1:
                        drain = ins
    if drain is None or not keep_ids:
        return
    si = drain.sync_info
    kept = [w for w in si.on_wait if w.id in keep_ids]
    if kept:
        si.on_wait = kept
        drain.sync_info = si


def _slim_drain_and_barrier(self, tick_clock, wait_clock):
    """Replacement for TileContext._drain_and_barrier: the NEFF here runs
    exactly once per load (run_bass_via_pjrt → single execute), so skip
    the semaphore re-zeroing and the end barriers entirely."""
    drain_inst = self.nc.sync.drain()
    wait_clock.add_sem_waits(
        drain_inst.ins, tile.ScopedClock({None: tick_clock.global_clock})
    )
    popped = self.nc._tile_sem_poison_stack.pop()
    assert popped is self._sem_poison


tile.TileContext._drain_and_barrier = _slim_drain_and_barrier


# ---------------------------------------------------------------------------
# Host-side routing: build the per-core slot structure.
# ---------------------------------------------------------------------------
def _plan(e: np.ndarray):
    """Return (slot_widths, per_core_slots) where per_core_slots[i] is a list
    of (expert_id, sample_indices) aligned with slot_widths (desc order)."""
    order = np.argsort(e, kind="stable")
    counts = np.bincount(e, minlength=max(E, int(e.max()) + 1 if len(e) else E))
    starts = np.concatenate([[0], np.cumsum(counts)])

    chunks = []  # (width, expert, indices)
    for ex in range(len(counts)):
        idx = order[starts[ex] : starts[ex + 1]]
        for pos in range(0, len(idx), MAX_W):
            sub = idx[pos : pos + MAX_W]
            chunks.append((len(sub), ex, sub))
    chunks.sort(key=lambda t: -t[0])

    per_core = [[] for _ in range(N_CORES)]
    for r in range(0, len(chunks), N_CORES):
        row = chunks[r : r + N_CORES]
        cores = range(N_CORES) if (r // N_CORES) % 2 == 0 else range(N_CORES - 1, -1, -1)
        for ch, core in zip(row, cores):
            per_core[core].append(ch)

    n_slots = max(len(s) for s in per_core)
    empty = np.zeros((0,), dtype=np.int64)
    for s in per_core:
        while len(s) < n_slots:
            s.append((0, 0, empty))
        s.sort(key=lambda t: -t[0])

    widths = [max(per_core[i][j][0] for i in range(N_CORES)) for j in range(n_slots)]
    widths = [max(w, 1) for w in widths]
    slots = [[(s[j][1], s[j][2]) for j in range(n_slots)] for s in per_core]
    return widths, slots


class _Layout:
    """Column layout shared by the graph builder and the host packer.

    data tensor (bf16 cols):
      [0, 2S)          b1 columns, f32 bitcast (col j = b1 of slot j)
      [2S, 2S+2NB)     b2 columns, f32 bitcast (col b, partition i = b2 of
                       bins[b][i])
      [HDR, ...)       per pair p: W1_p (HID cols, slot 2p on partitions
                       0:64, slot 2p+1 on 64:128) then xT_p (pw_p cols,
                       same stacking); pairs 0, 1, 2.. in order
      [W2_OFF, +S)     w2 columns in bin order (col slot_pos[j])
    Input DMA split: A = header + w2 + pair0 (sync), B = pair1 (scalar),
    C = pairs 2.. (sync).
    """

    def __init__(self, widths):
        S = len(widths)
        P = (S + 1) // 2
        self.widths = widths
        self.S, self.P = S, P
        self.NT = int(np.sum(widths))

        bins, bin_w = [], []
        self.slot_bin = [0] * S
        for j in range(S):
            for b in range(len(bins)):
                if bin_w[b] + widths[j] <= MAX_W:
                    bins[b].append(j)
                    bin_w[b] += widths[j]
                    self.slot_bin[j] = b
                    break
            else:
                self.slot_bin[j] = len(bins)
                bins.append([j])
                bin_w.append(widths[j])
        # the last bin completes last: keep its final slot solo so the
        # endgame mm2+sigmoid chain is as short as possible
        if len(bins[-1]) > 1 and len(bins) < 7:
            j = bins[-1].pop()
            bin_w[-1] -= widths[j]
            self.slot_bin[j] = len(bins)
            bins.append([j])
            bin_w.append(widths[j])
        self.bins, self.bin_w = bins, bin_w
        self.NB = len(bins)
        self.Mmax = max(len(bs) for bs in bins)

        # slot processing order = bin order, so bins complete (and their
        # mm2+sigmoid fire) sequentially instead of piling up at the end
        self.proc = [j for bs in bins for j in bs]

        self.slot_y_off = [0] * S  # column in y / position of slot's range
        self.slot_row = [0] * S  # row in y
        self.slot_pos = [0] * S  # w2 column
        self.bin_off = []
        off = pos = 0
        for b, bs in enumerate(bins):
            self.bin_off.append(off)
            for i, j in enumerate(bs):
                self.slot_y_off[j] = off
                self.slot_row[j] = i
                self.slot_pos[j] = pos
                off += widths[j]
                pos += 1
        assert off == self.NT

        # pairs follow the processing order: pair k stacks proc[2k] on
        # partitions 0:64 and proc[2k+1] on 64:128
        self.pair_of = {}
        self.hi_of = {}
        self.pairs = []
        for k in range(P):
            js = self.proc[2 * k : 2 * k + 2]
            self.pairs.append(js)
            for hi, j in enumerate(js):
                self.pair_of[j] = k
                self.hi_of[j] = hi
        self.pw = [
            max(widths[j] for j in js) + (max(widths[j] for j in js) & 1)
            for js in self.pairs
        ]

        self.HDR = 2 * S + 2 * self.NB
        self.pair_base = []
        c = self.HDR
        for k in range(P):
            self.pair_base.append(c)
            c += HID + self.pw[k]
        self.W2_OFF = c
        self.DCOLS = c + S + (S & 1)
        self.CUT1 = self.pair_base[1] if P > 1 else self.W2_OFF
        self.CUT2 = self.pair_base[2] if P > 2 else self.W2_OFF
        # y DMA chunks (bin ranges): ~thirds, the last bin always alone so
        # its DMA (the one the kernel-end waits on) is issued ASAP
        self.ychunks = []
        if self.NB == 1:
            self.ychunks = [(0, 1)]
        else:
            target = self.NT / 3
            cur = 0
            for b in range(self.NB - 1):
                if (
                    self.bin_off[b + 1] - self.bin_off[cur] >= target
                    or b == self.NB - 2
                ):
                    self.ychunks.append((cur, b + 1))
                    cur = b + 1
            self.ychunks.append((cur, self.NB))

    def chunk_cols(self, ch):
        s, e = self.ychunks[ch]
        c0 = self.bin_off[s]
        c1 = self.bin_off[e] if e < self.NB else self.NT
        return c0, c1

    def w1_cols(self, j):
        p = self.pair_of[j]
        return self.pair_base[p], self.pair_base[p] + HID

    def xt_cols(self, j):
        p = self.pair_of[j]
        c0 = self.pair_base[p] + HID
        return c0, c0 + self.widths[j]


# ---------------------------------------------------------------------------
# Device graph builder (shared by all cores).
# ---------------------------------------------------------------------------
def _build(L: _Layout):
    S, P, NB = L.S, L.P, L.NB
    widths = L.widths

    nc = bass.Bass("TRN2", target_bir_lowering=False, debug=False)
    data_e = nc.declare_dram_parameter("data", [128, L.DCOLS], BF16, isOutput=False)
    y_e = nc.declare_dram_parameter("y", [L.Mmax, L.NT], F32, isOutput=True)

    sigmoid = mybir.ActivationFunctionType.Sigmoid
    add = mybir.AluOpType.add
    amax = mybir.AluOpType.max

    out_dma_names = []
    with tile.TileContext(nc) as tc:
        with (
            tc.tile_pool(name="sb", bufs=1) as sb,
            tc.tile_pool(
                name="ps1", bufs=max(1, min(3, 8 - NB)), space="PSUM"
            ) as ps1,
            tc.tile_pool(name="ps2", bufs=1, space="PSUM") as ps2,
            tc.tile_pool(name="dummy", bufs=1) as dummy_pool,
        ):
            # Engine preloads during the input DMA window (all on garbage
            # SBUF, no data deps): ACT sigmoid table load, DVE first-op
            # cost, PE pipeline priming. The warmup matmuls rotate through
            # the same psum bufs the real mm1s use (PE executes in order).
            WARMUP = os.environ.get("K_WARMUP", "1") == "1"
            if WARMUP:
                warm = dummy_pool.tile([128, 512], BF16)
                warm_in = dummy_pool.tile([1, 16], F32)
                warm_y = dummy_pool.tile([1, 16], F32)
                warm_v = dummy_pool.tile([1, 16], F32)
                nc.gpsimd.memset(warm[:], 0.0)
                nc.gpsimd.memset(warm_in[:], 0.0)
                nc.scalar.activation(warm_y[:], warm_in[:], sigmoid)
                nc.vector.tensor_scalar(
                    warm_v[:], warm_in[:], 0.0, 0.0, add, amax
                )
                for _ in range(int(os.environ.get("K_NWARM", "4"))):
                    warm_ps = ps1.tile([HID, 448], F32, tag="p1")
                    nc.tensor.matmul(
                        warm_ps[:], warm[:, :128], warm[:, :448],
                        start=True, stop=True,
                    )

            dataA = sb.tile([128, L.CUT1], BF16)
            dataB = sb.tile([128, max(L.CUT2 - L.CUT1, 1)], BF16)
            dataC = sb.tile([128, max(L.DCOLS - L.CUT2, 1)], BF16)
            y_t = []
            for ch in range(len(L.ychunks)):
                c0, c1 = L.chunk_cols(ch)
                yt = sb.tile([L.Mmax, c1 - c0], F32, tag=f"y{ch}")
                y_t.append(yt)
            hbin = []
            for b in range(NB):
                hb = sb.tile([HID, L.bin_w[b]], BF16, tag=f"h{b}")
                hbin.append(hb)

            nc.sync.dma_start(dataA[:], data_e[:, : L.CUT1])
            if L.CUT2 > L.CUT1:
                nc.scalar.dma_start(dataB[:], data_e[:, L.CUT1 : L.CUT2])
            if L.DCOLS > L.CUT2:
                nc.sync.dma_start(dataC[:], data_e[:, L.CUT2 :])

            def dcols(c0, c1, r0=0, r1=128):
                if c1 <= L.CUT1:
                    return dataA[r0:r1, c0:c1]
                if c1 <= L.CUT2:
                    assert c0 >= L.CUT1
                    return dataB[r0:r1, c0 - L.CUT1 : c1 - L.CUT1]
                assert c0 >= L.CUT2
                return dataC[r0:r1, c0 - L.CUT2 : c1 - L.CUT2]

            def b1_ap(j):
                return dataA[:, 2 * j : 2 * j + 2].bitcast(F32)

            def b2_ap(b, m):
                c = 2 * S + 2 * b
                return dataA[0:m, c : c + 2].bitcast(F32)

            def yslice(r, c0, c1):
                for ch in range(len(L.ychunks)):
                    s0, s1 = L.chunk_cols(ch)
                    if c0 >= s0 and c1 <= s1:
                        return y_t[ch][0:r, c0 - s0 : c1 - s0]
                raise AssertionError("bin straddles y chunk")

            def mm1(j):
                r0 = 64 * L.hi_of[j]
                c0, c1 = L.xt_cols(j)
                w0, w1c = L.w1_cols(j)
                p1 = ps1.tile([HID, widths[j]], F32, tag="p1")
                nc.tensor.matmul(
                    p1[:],
                    dcols(w0, w1c, r0, r0 + 64),
                    dcols(c0, c1, r0, r0 + 64),
                    start=True,
                    stop=True,
                )
                return p1

            n_act_relu = int(os.environ.get("K_ACTRELU", "1"))
            act_relu = set(L.proc[:n_act_relu])
            relu_fn = mybir.ActivationFunctionType.Relu

            def relu(j, p1):
                b = L.slot_bin[j]
                c0 = L.slot_y_off[j] - L.bin_off[b]
                out = hbin[b][:, c0 : c0 + widths[j]]
                if j in act_relu:
                    # ACT is idle until the first sigmoid; offload the
                    # leading relu(s) there to unblock DVE earlier
                    nc.scalar.activation(out, p1[:], relu_fn, bias=b1_ap(j))
                else:
                    nc.vector.tensor_scalar(
                        out, p1[:], b1_ap(j), 0.0, add, amax
                    )

            bin_ps = []
            for b in range(NB):
                m = len(L.bins[b])
                bp = ps2.tile([m, L.bin_w[b]], F32, tag=f"bin{b}")
                bin_ps.append(bp)
            bin_left = [len(bs) for bs in L.bins]

            def finish_bin(b):
                m = len(L.bins[b])
                p0 = L.slot_pos[L.bins[b][0]]
                nc.tensor.matmul(
                    bin_ps[b][:],
                    dcols(L.W2_OFF + p0, L.W2_OFF + p0 + m),
                    hbin[b][:],
                    start=True,
                    stop=True,
                )
                c0 = L.bin_off[b]
                nc.scalar.activation(
                    yslice(m, c0, c0 + L.bin_w[b]),
                    bin_ps[b][:],
                    sigmoid,
                    bias=b2_ap(b, m),
                )

            chunk_emitted = [False] * len(L.ychunks)

            def emit_ready_chunks():
                for ch, (s, e) in enumerate(L.ychunks):
                    if chunk_emitted[ch]:
                        continue
                    if all(bin_left[bb] == 0 for bb in range(s, e)):
                        c0, c1 = L.chunk_cols(ch)
                        d = nc.sync.dma_start(y_e[:, c0:c1], y_t[ch][:])
                        out_dma_names.append(d.ins.name)
                        chunk_emitted[ch] = True

            def finish_slot(j):
                b = L.slot_bin[j]
                bin_left[b] -= 1
                if bin_left[b] == 0:
                    finish_bin(b)
                    emit_ready_chunks()

            # software-pipelined emission: mm1 of pair p+1 runs on PE while
            # DVE does relu of pair p; bin mm2s/sigmoids fire as bins fill.
            stage = []  # (j, p1)
            for p in range(P + 1):
                if p < P:
                    nxt = [(j, mm1(j)) for j in L.pairs[p]]
                else:
                    nxt = []
                for j, p1 in stage:
                    relu(j, p1)
                    finish_slot(j)
                stage = nxt

            emit_ready_chunks()
            assert all(chunk_emitted), "unemitted y chunk"

    _filter_drain_waits(nc, out_dma_names)
    _split_multi_waits(nc)
    return nc


# ---------------------------------------------------------------------------
# Entry point.
# ---------------------------------------------------------------------------
def _run(inputs, trace=False):
    x = np.asarray(inputs["x"], dtype=np.float32)
    num = np.asarray(inputs["num"])
    c = np.asarray(inputs["c"])
    W1 = np.asarray(inputs["W1"], dtype=np.float32)
    b1 = np.asarray(inputs["b1"], dtype=np.float32)
    W2 = np.asarray(inputs["W2"], dtype=np.float32)
    b2 = np.asarray(inputs["b2"], dtype=np.float32)

    B = x.shape[0]
    e = c[num].astype(np.int64)
    widths, slots = _plan(e)
    L = _Layout(widths)
    S = L.S

    x_bf = x.astype(NP_BF16)
    W1_bf = W1.astype(NP_BF16)
    W2_bf = W2.astype(NP_BF16)

    in_maps = []
    for core in range(N_CORES):
        data_c = np.zeros((128, L.DCOLS), dtype=NP_BF16)
        b1_c = np.zeros((128, S), dtype=np.float32)
        b2_c = np.zeros((128, L.NB), dtype=np.float32)
        for j in range(S):
            ex, idx = slots[core][j]
            r0 = 64 * L.hi_of[j]
            w0, w1c = L.w1_cols(j)
            c0, _ = L.xt_cols(j)
            if len(idx):
                data_c[r0 : r0 + 64, c0 : c0 + len(idx)] = x_bf[idx].T
            data_c[r0 : r0 + 64, w0:w1c] = W1_bf[ex]
            data_c[:, L.W2_OFF + L.slot_pos[j]] = W2_bf[ex, :, 0]
            b1_c[:, j] = b1[ex]
            b2_c[L.slot_row[j], L.slot_bin[j]] = b2[ex, 0]
        data_c[:, : 2 * S] = b1_c.view(NP_BF16)
        data_c[:, 2 * S : 2 * S + 2 * L.NB] = b2_c.view(NP_BF16)
        in_maps.append({"data": data_c})

    nc = _build(L)
    res = run_bass_kernel_spmd(nc, in_maps, list(range(N_CORES)), trace=trace)

    out = np.empty((B, 1), dtype=np.float32)
    for core in range(N_CORES):
        y_c = res.results[core]["y"]
        for j in range(S):
            ex, idx = slots[core][j]
            if len(idx):
                out[idx, 0] = y_c[
                    L.slot_row[j], L.slot_y_off[j] : L.slot_y_off[j] + len(idx)
                ]
    return out, res


def kernel(**inputs) -> np.ndarray:
    out, _ = _run(inputs, trace=False)
    return out


# revision 51
# speedup vs baseline: 1.0058x; 1.0058x over previous
"""MoE routing kernel for Trainium2 (8 NeuronCores).

Reference computation (B=16384, IN=64, HID=128, OUT=1, E=64, NMAP=1000):
    e = c[num]                                  # [B] expert id per sample
    h = relu(x @ W1[e] + b1[e])                 # [B, HID]
    y = sigmoid(h @ W2[e] + b2[e])              # [B, OUT]

Strategy: sort-by-expert dispatch on the host (the routing is pure
integer bookkeeping), dense per-expert matmuls on device. Each core gets
the same static slot structure (SPMD: one graph for all 8 cores); slot
widths are equalized across cores by snake-dealing the per-expert chunks
in descending size order, padding each slot to the max width over cores.

Device layout (per core): slots are paired onto the 128 SBUF partitions
— pair p puts slot 2p's x^T on partitions 0:64 and slot 2p+1's on
64:128. Full-width DMA, and the two K=64 matmuls of a pair run
concurrently in disjoint PE row groups. Slots are also first-fit packed
into "bins" of <=512 y columns: each bin is one PSUM bank, one
block-diagonal mm2 (lhsT = the bin's w2 columns), and one sigmoid.
All tensor data is bf16 (rel-err budget 2e-2); accumulation stays f32.

Per slot j (width Wj <= 512, pair p, bin b):
    mm1:   psum1[HID=128, Wj] = W1_j[64,128].T @ xT[64, Wj]      (PE)
    relu:  hbin_b[:, cj:cj+Wj] = bf16(max(psum1 + b1_j, 0))      (DVE)
Per bin b (M slots, width Wb <= 512):
    mm2:   psum2[M, Wb] = w2_bin[128,M].T @ hbin_b[128, Wb]      (PE)
    sig:   y[0:M, bin] = sigmoid(psum2 + b2_bin[M,1])            (ACT)
Slot j's outputs live in y[row_of_j_in_bin, its columns] (the
off-diagonal rows are garbage the host ignores).
"""

import os
import sys

if "/opt/trn_rl_repo" not in sys.path:
    sys.path.insert(0, "/opt/trn_rl_repo")

import numpy as np

import concourse.bass as bass
import concourse.mybir as mybir
from concourse import tile
from concourse.bass_utils import run_bass_kernel_spmd

N_CORES = 8
IN = 64
HID = 128
E = 64
MAX_W = 512  # moving-operand / PSUM-bank limit

BF16 = mybir.dt.bfloat16
F32 = mybir.dt.float32
NP_BF16 = mybir.dt.np(BF16)


# ---------------------------------------------------------------------------
# This container's walrus build rejects more than one sync wait per
# instruction ("Too many sync wait commands"). Post-pass over the lowered
# BIR: move the extra waits onto single-wait NOPs inserted just before the
# instruction on the same engine (program order makes this equivalent).
# ---------------------------------------------------------------------------
def _split_multi_waits(nc):
    ctr = 0
    for f in nc.m.functions:
        for blk in f.blocks:
            new_list = []
            for ins in blk.instructions:
                si = ins.sync_info
                if si is not None and si.on_wait and len(si.on_wait) > 1:
                    waits = list(si.on_wait)
                    for w in waits[:-1]:
                        ctr += 1
                        new_list.append(
                            mybir.InstNoOp(
                                name=f"waitsplit-{ctr}",
                                engine=ins.engine,
                                bass_nofuse=True,
                                sync_info=mybir.SyncInfo(
                                    on_wait=[w], on_update=[]
                                ),
                            )
                        )
                    si.on_wait = waits[-1:]
                    ins.sync_info = si
                new_list.append(ins)
            blk.instructions = new_list


def _filter_drain_waits(nc, out_dma_names):
    """The kernel-tail drain only needs to gate on the output DMAs'
    completion semaphores — every other wait Tile put on it is
    transitively implied. Fewer waits = fewer single-wait NOPs."""
    keep_ids = set()
    drain = None
    for f in nc.m.functions:
        for blk in f.blocks:
            for ins in blk.instructions:
                if ins.name in out_dma_names and ins.sync_info is not None:
                    for u in ins.sync_info.on_update:
                        keep_ids.add(u.id)
                if isinstance(ins, mybir.InstDrain):
                    si = ins.sync_info
                    if si is not None and len(si.on_wait) > 1:
                        drain = ins
    if drain is None or not keep_ids:
        return
    si = drain.sync_info
    kept = [w for w in si.on_wait if w.id in keep_ids]
    if kept:
        si.on_wait = kept
        drain.sync_info = si


def _slim_drain_and_barrier(self, tick_clock, wait_clock):
    """Replacement for TileContext._drain_and_barrier: the NEFF here runs
    exactly once per load (run_bass_via_pjrt → single execute), so skip
    the semaphore re-zeroing and the end barriers entirely."""
    drain_inst = self.nc.sync.drain()
    wait_clock.add_sem_waits(
        drain_inst.ins, tile.ScopedClock({None: tick_clock.global_clock})
    )
    popped = self.nc._tile_sem_poison_stack.pop()
    assert popped is self._sem_poison


tile.TileContext._drain_and_barrier = _slim_drain_and_barrier


# ---------------------------------------------------------------------------
# Host-side routing: build the per-core slot structure.
# ---------------------------------------------------------------------------
def _plan(e: np.ndarray):
    """Return (slot_widths, per_core_slots) where per_core_slots[i] is a list
    of (expert_id, sample_indices) aligned with slot_widths (desc order)."""
    order = np.argsort(e, kind="stable")
    counts = np.bincount(e, minlength=max(E, int(e.max()) + 1 if len(e) else E))
    starts = np.concatenate([[0], np.cumsum(counts)])

    chunks = []  # (width, expert, indices)
    for ex in range(len(counts)):
        idx = order[starts[ex] : starts[ex + 1]]
        for pos in range(0, len(idx), MAX_W):
            sub = idx[pos : pos + MAX_W]
            chunks.append((len(sub), ex, sub))
    chunks.sort(key=lambda t: -t[0])

    per_core = [[] for _ in range(N_CORES)]
    for r in range(0, len(chunks), N_CORES):
        row = chunks[r : r + N_CORES]
        cores = range(N_CORES) if (r // N_CORES) % 2 == 0 else range(N_CORES - 1, -1, -1)
        for ch, core in zip(row, cores):
            per_core[core].append(ch)

    n_slots = max(len(s) for s in per_core)
    empty = np.zeros((0,), dtype=np.int64)
    for s in per_core:
        while len(s) < n_slots:
            s.append((0, 0, empty))
        s.sort(key=lambda t: -t[0])

    widths = [max(per_core[i][j][0] for i in range(N_CORES)) for j in range(n_slots)]
    widths = [max(w, 1) for w in widths]
    slots = [[(s[j][1], s[j][2]) for j in range(n_slots)] for s in per_core]
    return widths, slots


class _Layout:
    """Column layout shared by the graph builder and the host packer.

    data tensor (bf16 cols):
      [0, 2S)          b1 columns, f32 bitcast (col j = b1 of slot j)
      [2S, 2S+2NB)     b2 columns, f32 bitcast (col b, partition i = b2 of
                       bins[b][i])
      [HDR, ...)       per pair p: W1_p (HID cols, slot 2p on partitions
                       0:64, slot 2p+1 on 64:128) then xT_p (pw_p cols,
                       same stacking); pairs 0, 1, 2.. in order
      [W2_OFF, +S)     w2 columns in bin order (col slot_pos[j])
    Input DMA split: A = header + w2 + pair0 (sync), B = pair1 (scalar),
    C = pairs 2.. (sync).
    """

    def __init__(self, widths):
        S = len(widths)
        P = (S + 1) // 2
        self.widths = widths
        self.S, self.P = S, P
        self.NT = int(np.sum(widths))

        bins, bin_w = [], []
        self.slot_bin = [0] * S
        for j in range(S):
            for b in range(len(bins)):
                if bin_w[b] + widths[j] <= MAX_W:
                    bins[b].append(j)
                    bin_w[b] += widths[j]
                    self.slot_bin[j] = b
                    break
            else:
                self.slot_bin[j] = len(bins)
                bins.append([j])
                bin_w.append(widths[j])
        # the last bin completes last: keep its final slot solo so the
        # endgame mm2+sigmoid chain is as short as possible
        if (
            os.environ.get("K_SOLOBIN", "0") == "1"
            and len(bins[-1]) > 1
            and len(bins) < 7
        ):
            j = bins[-1].pop()
            bin_w[-1] -= widths[j]
            self.slot_bin[j] = len(bins)
            bins.append([j])
            bin_w.append(widths[j])
        self.bins, self.bin_w = bins, bin_w
        self.NB = len(bins)
        self.Mmax = max(len(bs) for bs in bins)

        # slot processing order = bin order, so bins complete (and their
        # mm2+sigmoid fire) sequentially instead of piling up at the end
        self.proc = [j for bs in bins for j in bs]

        self.slot_y_off = [0] * S  # column in y / position of slot's range
        self.slot_row = [0] * S  # row in y
        self.slot_pos = [0] * S  # w2 column
        self.bin_off = []
        off = pos = 0
        for b, bs in enumerate(bins):
            self.bin_off.append(off)
            for i, j in enumerate(bs):
                self.slot_y_off[j] = off
                self.slot_row[j] = i
                self.slot_pos[j] = pos
                off += widths[j]
                pos += 1
        assert off == self.NT

        # pairs follow the processing order: pair k stacks proc[2k] on
        # partitions 0:64 and proc[2k+1] on 64:128
        self.pair_of = {}
        self.hi_of = {}
        self.pairs = []
        for k in range(P):
            js = self.proc[2 * k : 2 * k + 2]
            self.pairs.append(js)
            for hi, j in enumerate(js):
                self.pair_of[j] = k
                self.hi_of[j] = hi
        self.pw = [
            max(widths[j] for j in js) + (max(widths[j] for j in js) & 1)
            for js in self.pairs
        ]

        self.HDR = 2 * S + 2 * self.NB
        self.pair_base = []
        c = self.HDR
        for k in range(P):
            self.pair_base.append(c)
            c += HID + self.pw[k]
        self.W2_OFF = c
        self.DCOLS = c + S + (S & 1)
        self.CUT1 = self.pair_base[1] if P > 1 else self.W2_OFF
        self.CUT2 = self.pair_base[2] if P > 2 else self.W2_OFF
        # y DMA chunks (bin ranges): ~thirds, the last bin always alone so
        # its DMA (the one the kernel-end waits on) is issued ASAP
        self.ychunks = []
        if self.NB == 1:
            self.ychunks = [(0, 1)]
        else:
            target = self.NT / int(os.environ.get("K_YCHUNKS", "2"))
            cur = 0
            for b in range(self.NB - 1):
                if (
                    self.bin_off[b + 1] - self.bin_off[cur] >= target
                    or b == self.NB - 2
                ):
                    self.ychunks.append((cur, b + 1))
                    cur = b + 1
            self.ychunks.append((cur, self.NB))

    def chunk_cols(self, ch):
        s, e = self.ychunks[ch]
        c0 = self.bin_off[s]
        c1 = self.bin_off[e] if e < self.NB else self.NT
        return c0, c1

    def w1_cols(self, j):
        p = self.pair_of[j]
        return self.pair_base[p], self.pair_base[p] + HID

    def xt_cols(self, j):
        p = self.pair_of[j]
        c0 = self.pair_base[p] + HID
        return c0, c0 + self.widths[j]


# ---------------------------------------------------------------------------
# Device graph builder (shared by all cores).
# ---------------------------------------------------------------------------
def _build(L: _Layout):
    S, P, NB = L.S, L.P, L.NB
    widths = L.widths

    nc = bass.Bass("TRN2", target_bir_lowering=False, debug=False)
    data_e = nc.declare_dram_parameter("data", [128, L.DCOLS], BF16, isOutput=False)
    y_e = nc.declare_dram_parameter("y", [L.Mmax, L.NT], F32, isOutput=True)

    sigmoid = mybir.ActivationFunctionType.Sigmoid
    add = mybir.AluOpType.add
    amax = mybir.AluOpType.max

    out_dma_names = []
    with tile.TileContext(nc) as tc:
        with (
            tc.tile_pool(name="sb", bufs=1) as sb,
            tc.tile_pool(
                name="ps1", bufs=max(1, min(3, 8 - NB)), space="PSUM"
            ) as ps1,
            tc.tile_pool(name="ps2", bufs=1, space="PSUM") as ps2,
            tc.tile_pool(name="dummy", bufs=1) as dummy_pool,
        ):
            # Engine preloads during the input DMA window (all on garbage
            # SBUF, no data deps): ACT sigmoid table load, DVE first-op
            # cost, PE pipeline priming. The warmup matmuls rotate through
            # the same psum bufs the real mm1s use (PE executes in order).
            WARMUP = os.environ.get("K_WARMUP", "1") == "1"
            if WARMUP:
                warm = dummy_pool.tile([128, 512], BF16)
                warm_in = dummy_pool.tile([1, 16], F32)
                warm_y = dummy_pool.tile([1, 16], F32)
                warm_v = dummy_pool.tile([1, 16], F32)
                nc.gpsimd.memset(warm[:], 0.0)
                nc.gpsimd.memset(warm_in[:], 0.0)
                nc.scalar.activation(warm_y[:], warm_in[:], sigmoid)
                nc.vector.tensor_scalar(
                    warm_v[:], warm_in[:], 0.0, 0.0, add, amax
                )
                for _ in range(int(os.environ.get("K_NWARM", "4"))):
                    warm_ps = ps1.tile([HID, 448], F32, tag="p1")
                    nc.tensor.matmul(
                        warm_ps[:], warm[:, :128], warm[:, :448],
                        start=True, stop=True,
                    )

            dataA = sb.tile([128, L.CUT1], BF16)
            dataB = sb.tile([128, max(L.CUT2 - L.CUT1, 1)], BF16)
            dataC = sb.tile([128, max(L.DCOLS - L.CUT2, 1)], BF16)
            y_t = []
            for ch in range(len(L.ychunks)):
                c0, c1 = L.chunk_cols(ch)
                yt = sb.tile([L.Mmax, c1 - c0], F32, tag=f"y{ch}")
                y_t.append(yt)
            hbin = []
            for b in range(NB):
                hb = sb.tile([HID, L.bin_w[b]], BF16, tag=f"h{b}")
                hbin.append(hb)

            nc.sync.dma_start(dataA[:], data_e[:, : L.CUT1])
            if L.CUT2 > L.CUT1:
                nc.scalar.dma_start(dataB[:], data_e[:, L.CUT1 : L.CUT2])
            if L.DCOLS > L.CUT2:
                nc.sync.dma_start(dataC[:], data_e[:, L.CUT2 :])

            def dcols(c0, c1, r0=0, r1=128):
                if c1 <= L.CUT1:
                    return dataA[r0:r1, c0:c1]
                if c1 <= L.CUT2:
                    assert c0 >= L.CUT1
                    return dataB[r0:r1, c0 - L.CUT1 : c1 - L.CUT1]
                assert c0 >= L.CUT2
                return dataC[r0:r1, c0 - L.CUT2 : c1 - L.CUT2]

            def b1_ap(j):
                return dataA[:, 2 * j : 2 * j + 2].bitcast(F32)

            def b2_ap(b, m):
                c = 2 * S + 2 * b
                return dataA[0:m, c : c + 2].bitcast(F32)

            def yslice(r, c0, c1):
                for ch in range(len(L.ychunks)):
                    s0, s1 = L.chunk_cols(ch)
                    if c0 >= s0 and c1 <= s1:
                        return y_t[ch][0:r, c0 - s0 : c1 - s0]
                raise AssertionError("bin straddles y chunk")

            def mm1(j):
                r0 = 64 * L.hi_of[j]
                c0, c1 = L.xt_cols(j)
                w0, w1c = L.w1_cols(j)
                p1 = ps1.tile([HID, widths[j]], F32, tag="p1")
                nc.tensor.matmul(
                    p1[:],
                    dcols(w0, w1c, r0, r0 + 64),
                    dcols(c0, c1, r0, r0 + 64),
                    start=True,
                    stop=True,
                )
                return p1

            n_act_relu = int(os.environ.get("K_ACTRELU", "1"))
            act_relu = set(L.proc[:n_act_relu])
            relu_fn = mybir.ActivationFunctionType.Relu

            def relu(j, p1):
                b = L.slot_bin[j]
                c0 = L.slot_y_off[j] - L.bin_off[b]
                out = hbin[b][:, c0 : c0 + widths[j]]
                if j in act_relu:
                    # ACT is idle until the first sigmoid; offload the
                    # leading relu(s) there to unblock DVE earlier
                    nc.scalar.activation(out, p1[:], relu_fn, bias=b1_ap(j))
                else:
                    nc.vector.tensor_scalar(
                        out, p1[:], b1_ap(j), 0.0, add, amax
                    )

            bin_ps = []
            for b in range(NB):
                m = len(L.bins[b])
                bp = ps2.tile([m, L.bin_w[b]], F32, tag=f"bin{b}")
                bin_ps.append(bp)
            bin_left = [len(bs) for bs in L.bins]

            def finish_bin(b):
                m = len(L.bins[b])
                p0 = L.slot_pos[L.bins[b][0]]
                nc.tensor.matmul(
                    bin_ps[b][:],
                    dcols(L.W2_OFF + p0, L.W2_OFF + p0 + m),
                    hbin[b][:],
                    start=True,
                    stop=True,
                )
                c0 = L.bin_off[b]
                nc.scalar.activation(
                    yslice(m, c0, c0 + L.bin_w[b]),
                    bin_ps[b][:],
                    sigmoid,
                    bias=b2_ap(b, m),
                )

            chunk_emitted = [False] * len(L.ychunks)

            def emit_ready_chunks():
                for ch, (s, e) in enumerate(L.ychunks):
                    if chunk_emitted[ch]:
                        continue
                    if all(bin_left[bb] == 0 for bb in range(s, e)):
                        c0, c1 = L.chunk_cols(ch)
                        # the final chunk leaves straight from the ACT
                        # engine's queue — no cross-engine semaphore hop
                        # after the last sigmoid
                        eng = (
                            nc.scalar
                            if ch == len(L.ychunks) - 1
                            and os.environ.get("K_YSCALAR", "0") == "1"
                            else nc.sync
                        )
                        d = eng.dma_start(y_e[:, c0:c1], y_t[ch][:])
                        out_dma_names.append(d.ins.name)
                        chunk_emitted[ch] = True

            def finish_slot(j):
                b = L.slot_bin[j]
                bin_left[b] -= 1
                if bin_left[b] == 0:
                    finish_bin(b)
                    emit_ready_chunks()

            # software-pipelined emission: mm1 of pair p+1 runs on PE while
            # DVE does relu of pair p; bin mm2s/sigmoids fire as bins fill.
            stage = []  # (j, p1)
            for p in range(P + 1):
                if p < P:
                    nxt = [(j, mm1(j)) for j in L.pairs[p]]
                else:
                    nxt = []
                for j, p1 in stage:
                    relu(j, p1)
                    finish_slot(j)
                stage = nxt

            emit_ready_chunks()
            assert all(chunk_emitted), "unemitted y chunk"

    _filter_drain_waits(nc, out_dma_names)
    _split_multi_waits(nc)
    return nc


# ---------------------------------------------------------------------------
# Entry point.
# ---------------------------------------------------------------------------
def _run(inputs, trace=False):
    x = np.asarray(inputs["x"], dtype=np.float32)
    num = np.asarray(inputs["num"])
    c = np.asarray(inputs["c"])
    W1 = np.asarray(inputs["W1"], dtype=np.float32)
    b1 = np.asarray(inputs["b1"], dtype=np.float32)
    W2 = np.asarray(inputs["W2"], dtype=np.float32)
    b2 = np.asarray(inputs["b2"], dtype=np.float32)

    B = x.shape[0]
    e = c[num].astype(np.int64)
    widths, slots = _plan(e)
    L = _Layout(widths)
    S = L.S

    x_bf = x.astype(NP_BF16)
    W1_bf = W1.astype(NP_BF16)
    W2_bf = W2.astype(NP_BF16)

    in_maps = []
    for core in range(N_CORES):
        data_c = np.zeros((128, L.DCOLS), dtype=NP_BF16)
        b1_c = np.zeros((128, S), dtype=np.float32)
        b2_c = np.zeros((128, L.NB), dtype=np.float32)
        for j in range(S):
            ex, idx = slots[core][j]
            r0 = 64 * L.hi_of[j]
            w0, w1c = L.w1_cols(j)
            c0, _ = L.xt_cols(j)
            if len(idx):
                data_c[r0 : r0 + 64, c0 : c0 + len(idx)] = x_bf[idx].T
            data_c[r0 : r0 + 64, w0:w1c] = W1_bf[ex]
            data_c[:, L.W2_OFF + L.slot_pos[j]] = W2_bf[ex, :, 0]
            b1_c[:, j] = b1[ex]
            b2_c[L.slot_row[j], L.slot_bin[j]] = b2[ex, 0]
        data_c[:, : 2 * S] = b1_c.view(NP_BF16)
        data_c[:, 2 * S : 2 * S + 2 * L.NB] = b2_c.view(NP_BF16)
        in_maps.append({"data": data_c})

    nc = _build(L)
    res = run_bass_kernel_spmd(nc, in_maps, list(range(N_CORES)), trace=trace)

    out = np.empty((B, 1), dtype=np.float32)
    for core in range(N_CORES):
        y_c = res.results[core]["y"]
        for j in range(S):
            ex, idx = slots[core][j]
            if len(idx):
                out[idx, 0] = y_c[
                    L.slot_row[j], L.slot_y_off[j] : L.slot_y_off[j] + len(idx)
                ]
    return out, res


def kernel(**inputs) -> np.ndarray:
    out, _ = _run(inputs, trace=False)
    return out


# revision 60
# speedup vs baseline: 1.0769x; 1.0707x over previous
"""MoE routing kernel for Trainium2 (8 NeuronCores).

Reference computation (B=16384, IN=64, HID=128, OUT=1, E=64, NMAP=1000):
    e = c[num]                                  # [B] expert id per sample
    h = relu(x @ W1[e] + b1[e])                 # [B, HID]
    y = sigmoid(h @ W2[e] + b2[e])              # [B, OUT]

Strategy: sort-by-expert dispatch on the host (the routing is pure
integer bookkeeping), dense per-expert matmuls on device. Each core gets
the same static slot structure (SPMD: one graph for all 8 cores); slot
widths are equalized across cores by snake-dealing the per-expert chunks
in descending size order, padding each slot to the max width over cores.

Device layout (per core): slots are paired onto the 128 SBUF partitions
— pair p puts slot 2p's x^T on partitions 0:64 and slot 2p+1's on
64:128. Full-width DMA, and the two K=64 matmuls of a pair run
concurrently in disjoint PE row groups. Slots are also first-fit packed
into "bins" of <=512 y columns: each bin is one PSUM bank, one
block-diagonal mm2 (lhsT = the bin's w2 columns), and one sigmoid.
All tensor data is bf16 (rel-err budget 2e-2); accumulation stays f32.

Per slot j (width Wj <= 512, pair p, bin b):
    mm1:   psum1[HID=128, Wj] = W1_j[64,128].T @ xT[64, Wj]      (PE)
    relu:  hbin_b[:, cj:cj+Wj] = bf16(max(psum1 + b1_j, 0))      (DVE)
Per bin b (M slots, width Wb <= 512):
    mm2:   psum2[M, Wb] = w2_bin[128,M].T @ hbin_b[128, Wb]      (PE)
    sig:   y[0:M, bin] = sigmoid(psum2 + b2_bin[M,1])            (ACT)
Slot j's outputs live in y[row_of_j_in_bin, its columns] (the
off-diagonal rows are garbage the host ignores).
"""

import os
import sys

if "/opt/trn_rl_repo" not in sys.path:
    sys.path.insert(0, "/opt/trn_rl_repo")

import numpy as np

import concourse.bass as bass
import concourse.mybir as mybir
from concourse import tile
from concourse.bass_utils import run_bass_kernel_spmd

N_CORES = 8
IN = 64
HID = 128
E = 64
MAX_W = 512  # moving-operand / PSUM-bank limit

BF16 = mybir.dt.bfloat16
F32 = mybir.dt.float32
NP_BF16 = mybir.dt.np(BF16)


# ---------------------------------------------------------------------------
# This container's walrus build rejects more than one sync wait per
# instruction ("Too many sync wait commands"). Post-pass over the lowered
# BIR: move the extra waits onto single-wait NOPs inserted just before the
# instruction on the same engine (program order makes this equivalent).
# ---------------------------------------------------------------------------
def _split_multi_waits(nc):
    ctr = 0
    for f in nc.m.functions:
        for blk in f.blocks:
            new_list = []
            for ins in blk.instructions:
                si = ins.sync_info
                if si is not None and si.on_wait and len(si.on_wait) > 1:
                    waits = list(si.on_wait)
                    for w in waits[:-1]:
                        ctr += 1
                        new_list.append(
                            mybir.InstNoOp(
                                name=f"waitsplit-{ctr}",
                                engine=ins.engine,
                                bass_nofuse=True,
                                sync_info=mybir.SyncInfo(
                                    on_wait=[w], on_update=[]
                                ),
                            )
                        )
                    si.on_wait = waits[-1:]
                    ins.sync_info = si
                new_list.append(ins)
            blk.instructions = new_list


def _filter_drain_waits(nc, out_dma_names):
    """The kernel-tail drain only needs to gate on the output DMAs'
    completion semaphores — every other wait Tile put on it is
    transitively implied. Fewer waits = fewer single-wait NOPs."""
    keep_ids = set()
    drain = None
    for f in nc.m.functions:
        for blk in f.blocks:
            for ins in blk.instructions:
                if ins.name in out_dma_names and ins.sync_info is not None:
                    for u in ins.sync_info.on_update:
                        keep_ids.add(u.id)
                if isinstance(ins, mybir.InstDrain):
                    si = ins.sync_info
                    if si is not None and len(si.on_wait) > 1:
                        drain = ins
    if drain is None or not keep_ids:
        return
    si = drain.sync_info
    kept = [w for w in si.on_wait if w.id in keep_ids]
    if kept:
        si.on_wait = kept
        drain.sync_info = si


def _slim_drain_and_barrier(self, tick_clock, wait_clock):
    """Replacement for TileContext._drain_and_barrier: the NEFF here runs
    exactly once per load (run_bass_via_pjrt → single execute), so skip
    the semaphore re-zeroing and the end barriers entirely."""
    drain_inst = self.nc.sync.drain()
    wait_clock.add_sem_waits(
        drain_inst.ins, tile.ScopedClock({None: tick_clock.global_clock})
    )
    popped = self.nc._tile_sem_poison_stack.pop()
    assert popped is self._sem_poison


tile.TileContext._drain_and_barrier = _slim_drain_and_barrier


# ---------------------------------------------------------------------------
# Host-side routing: build the per-core slot structure.
# ---------------------------------------------------------------------------
def _plan(e: np.ndarray):
    """Return (slot_widths, per_core_slots) where per_core_slots[i] is a list
    of (expert_id, sample_indices) aligned with slot_widths (desc order)."""
    order = np.argsort(e, kind="stable")
    counts = np.bincount(e, minlength=max(E, int(e.max()) + 1 if len(e) else E))
    starts = np.concatenate([[0], np.cumsum(counts)])

    chunks = []  # (width, expert, indices)
    for ex in range(len(counts)):
        idx = order[starts[ex] : starts[ex + 1]]
        for pos in range(0, len(idx), MAX_W):
            sub = idx[pos : pos + MAX_W]
            chunks.append((len(sub), ex, sub))
    chunks.sort(key=lambda t: -t[0])

    per_core = [[] for _ in range(N_CORES)]
    for r in range(0, len(chunks), N_CORES):
        row = chunks[r : r + N_CORES]
        cores = range(N_CORES) if (r // N_CORES) % 2 == 0 else range(N_CORES - 1, -1, -1)
        for ch, core in zip(row, cores):
            per_core[core].append(ch)

    n_slots = max(len(s) for s in per_core)
    empty = np.zeros((0,), dtype=np.int64)
    for s in per_core:
        while len(s) < n_slots:
            s.append((0, 0, empty))
        s.sort(key=lambda t: -t[0])

    widths = [max(per_core[i][j][0] for i in range(N_CORES)) for j in range(n_slots)]
    widths = [max(w, 1) for w in widths]
    slots = [[(s[j][1], s[j][2]) for j in range(n_slots)] for s in per_core]
    return widths, slots


class _Layout:
    """Column layout shared by the graph builder and the host packer.

    data tensor (bf16 cols):
      [0, 2S)          b1 columns, f32 bitcast (col j = b1 of slot j)
      [2S, 2S+2NB)     b2 columns, f32 bitcast (col b, partition i = b2 of
                       bins[b][i])
      [HDR, ...)       per pair p: W1_p (HID cols, slot 2p on partitions
                       0:64, slot 2p+1 on 64:128) then xT_p (pw_p cols,
                       same stacking); pairs 0, 1, 2.. in order
      [W2_OFF, +S)     w2 columns in bin order (col slot_pos[j])
    Input DMA split: A = header + w2 + pair0 (sync), B = pair1 (scalar),
    C = pairs 2.. (sync).
    """

    def __init__(self, widths):
        S = len(widths)
        P = (S + 1) // 2
        self.widths = widths
        self.S, self.P = S, P
        self.NT = int(np.sum(widths))

        bins, bin_w = [], []
        self.slot_bin = [0] * S
        for j in range(S):
            for b in range(len(bins)):
                if bin_w[b] + widths[j] <= MAX_W:
                    bins[b].append(j)
                    bin_w[b] += widths[j]
                    self.slot_bin[j] = b
                    break
            else:
                self.slot_bin[j] = len(bins)
                bins.append([j])
                bin_w.append(widths[j])
        # the last bin completes last: keep its final slot solo so the
        # endgame mm2+sigmoid chain is as short as possible
        if (
            os.environ.get("K_SOLOBIN", "0") == "1"
            and len(bins[-1]) > 1
            and len(bins) < 7
        ):
            j = bins[-1].pop()
            bin_w[-1] -= widths[j]
            self.slot_bin[j] = len(bins)
            bins.append([j])
            bin_w.append(widths[j])
        self.bins, self.bin_w = bins, bin_w
        self.NB = len(bins)
        self.Mmax = max(len(bs) for bs in bins)

        # slot processing order = bin order, so bins complete (and their
        # mm2+sigmoid fire) sequentially instead of piling up at the end
        self.proc = [j for bs in bins for j in bs]

        # banks: two bins share one PSUM bank — the second bin's mm2
        # lands at partition 32 (PE col-group tiling), so the paired
        # mm2s overlap on the PE and one sigmoid covers both bins.
        # The last bin stays solo so the final y DMA stays small.
        self.bank_of = [0] * self.NB
        self.rowbase = [0] * self.NB
        self.banks = []  # list of [bin ids]
        b = 0
        while b < self.NB:
            if b + 2 < self.NB or (b + 2 == self.NB and self.NB % 2 == 0):
                pair = [b, b + 1]
            else:
                pair = [b]
            for i, bb in enumerate(pair):
                self.bank_of[bb] = len(self.banks)
                self.rowbase[bb] = 32 * i
            self.banks.append(pair)
            b += len(pair)
        self.NBANK = len(self.banks)
        self.bank_w = [
            max(bin_w[bb] for bb in pair) for pair in self.banks
        ]

        self.slot_y_off = [0] * S  # column in y / position of slot's range
        self.slot_row = [0] * S  # row in y
        self.slot_pos = [0] * S  # w2 column
        self.bin_off = []
        off = pos = 0
        for b, bs in enumerate(bins):
            self.bin_off.append(off)
            for i, j in enumerate(bs):
                self.slot_y_off[j] = off
                self.slot_row[j] = i
                self.slot_pos[j] = pos
                off += widths[j]
                pos += 1
        assert off == self.NT

        # pairs follow the processing order: pair k stacks proc[2k] on
        # partitions 0:64 and proc[2k+1] on 64:128
        self.pair_of = {}
        self.hi_of = {}
        self.pairs = []
        for k in range(P):
            js = self.proc[2 * k : 2 * k + 2]
            self.pairs.append(js)
            for hi, j in enumerate(js):
                self.pair_of[j] = k
                self.hi_of[j] = hi
        self.pw = [
            max(widths[j] for j in js) + (max(widths[j] for j in js) & 1)
            for js in self.pairs
        ]

        # per-bank psum/sbuf row extent (bins at partition 0 and 32)
        self.bank_rows = [
            self.rowbase[pair[-1]] + len(bins[pair[-1]]) for pair in self.banks
        ]

        self.HDR = 2 * S + 2 * self.NBANK
        self.pair_base = []
        c = self.HDR
        for k in range(P):
            self.pair_base.append(c)
            c += HID + self.pw[k]
        self.W2_OFF = c
        self.DCOLS = c + S + (S & 1)
        self.CUT1 = self.pair_base[1] if P > 1 else self.W2_OFF
        self.CUT2 = self.pair_base[2] if P > 2 else self.W2_OFF

    def w1_cols(self, j):
        p = self.pair_of[j]
        return self.pair_base[p], self.pair_base[p] + HID

    def xt_cols(self, j):
        p = self.pair_of[j]
        c0 = self.pair_base[p] + HID
        return c0, c0 + self.widths[j]


# ---------------------------------------------------------------------------
# Device graph builder (shared by all cores).
# ---------------------------------------------------------------------------
def _build(L: _Layout):
    S, P, NB = L.S, L.P, L.NB
    widths = L.widths

    nc = bass.Bass("TRN2", target_bir_lowering=False, debug=False)
    data_e = nc.declare_dram_parameter("data", [128, L.DCOLS], BF16, isOutput=False)
    y_e = nc.declare_dram_parameter("y", [L.Mmax, L.NT], F32, isOutput=True)

    sigmoid = mybir.ActivationFunctionType.Sigmoid
    add = mybir.AluOpType.add
    amax = mybir.AluOpType.max

    out_dma_names = []
    with tile.TileContext(nc) as tc:
        with (
            tc.tile_pool(name="sb", bufs=1) as sb,
            tc.tile_pool(
                name="ps1", bufs=max(1, min(5, 8 - L.NBANK)), space="PSUM"
            ) as ps1,
            tc.tile_pool(name="ps2", bufs=1, space="PSUM") as ps2,
            tc.tile_pool(name="dummy", bufs=1) as dummy_pool,
        ):
            # Engine preloads during the input DMA window (all on garbage
            # SBUF, no data deps): ACT sigmoid table load, DVE first-op
            # cost, PE pipeline priming. The warmup matmuls rotate through
            # the same psum bufs the real mm1s use (PE executes in order).
            WARMUP = os.environ.get("K_WARMUP", "1") == "1"
            if WARMUP:
                warm = dummy_pool.tile([128, 512], BF16)
                warm_in = dummy_pool.tile([1, 16], F32)
                warm_y = dummy_pool.tile([1, 16], F32)
                warm_v = dummy_pool.tile([1, 16], F32)
                nc.gpsimd.memset(warm[:], 0.0)
                nc.gpsimd.memset(warm_in[:], 0.0)
                nc.scalar.activation(warm_y[:], warm_in[:], sigmoid)
                nc.vector.tensor_scalar(
                    warm_v[:], warm_in[:], 0.0, 0.0, add, amax
                )
                for _ in range(int(os.environ.get("K_NWARM", "4"))):
                    warm_ps = ps1.tile([HID, 448], F32, tag="p1")
                    nc.tensor.matmul(
                        warm_ps[:], warm[:, :128], warm[:, :448],
                        start=True, stop=True,
                    )

            dataA = sb.tile([128, L.CUT1], BF16)
            dataB = sb.tile([128, max(L.CUT2 - L.CUT1, 1)], BF16)
            dataC = sb.tile([128, max(L.DCOLS - L.CUT2, 1)], BF16)
            y_t = []
            for k in range(L.NBANK):
                yt = sb.tile([L.bank_rows[k], L.bank_w[k]], F32, tag=f"y{k}")
                y_t.append(yt)
            hbin = []
            for b in range(NB):
                hb = sb.tile([HID, L.bin_w[b]], BF16, tag=f"h{b}")
                hbin.append(hb)

            nc.sync.dma_start(dataA[:], data_e[:, : L.CUT1])
            if L.CUT2 > L.CUT1:
                nc.scalar.dma_start(dataB[:], data_e[:, L.CUT1 : L.CUT2])
            if L.DCOLS > L.CUT2:
                nc.sync.dma_start(dataC[:], data_e[:, L.CUT2 :])

            def dcols(c0, c1, r0=0, r1=128):
                if c1 <= L.CUT1:
                    return dataA[r0:r1, c0:c1]
                if c1 <= L.CUT2:
                    assert c0 >= L.CUT1
                    return dataB[r0:r1, c0 - L.CUT1 : c1 - L.CUT1]
                assert c0 >= L.CUT2
                return dataC[r0:r1, c0 - L.CUT2 : c1 - L.CUT2]

            def b1_ap(j):
                return dataA[:, 2 * j : 2 * j + 2].bitcast(F32)

            def b2_ap(k, rows):
                c = 2 * S + 2 * k
                return dataA[0:rows, c : c + 2].bitcast(F32)

            def mm1(j):
                r0 = 64 * L.hi_of[j]
                c0, c1 = L.xt_cols(j)
                w0, w1c = L.w1_cols(j)
                p1 = ps1.tile([HID, widths[j]], F32, tag="p1")
                nc.tensor.matmul(
                    p1[:],
                    dcols(w0, w1c, r0, r0 + 64),
                    dcols(c0, c1, r0, r0 + 64),
                    start=True,
                    stop=True,
                )
                return p1

            n_act_relu = int(os.environ.get("K_ACTRELU", "1"))
            act_relu = set(L.proc[:n_act_relu])
            relu_fn = mybir.ActivationFunctionType.Relu

            def relu(j, p1):
                b = L.slot_bin[j]
                c0 = L.slot_y_off[j] - L.bin_off[b]
                out = hbin[b][:, c0 : c0 + widths[j]]
                if j in act_relu:
                    # ACT is idle until the first sigmoid; offload the
                    # leading relu(s) there to unblock DVE earlier
                    nc.scalar.activation(out, p1[:], relu_fn, bias=b1_ap(j))
                else:
                    nc.vector.tensor_scalar(
                        out, p1[:], b1_ap(j), 0.0, add, amax
                    )

            bank_ps = []
            for k in range(L.NBANK):
                bp = ps2.tile([L.bank_rows[k], L.bank_w[k]], F32, tag=f"bk{k}")
                bank_ps.append(bp)
            bin_left = [len(bs) for bs in L.bins]
            bank_left = [len(pair) for pair in L.banks]

            def mm2_bin(b):
                m = len(L.bins[b])
                k = L.bank_of[b]
                r0 = L.rowbase[b]
                p0 = L.slot_pos[L.bins[b][0]]
                kwargs = {}
                if r0:
                    kwargs["tile_position"] = (0, r0)
                nc.tensor.matmul(
                    bank_ps[k][r0 : r0 + m, 0 : L.bin_w[b]],
                    dcols(L.W2_OFF + p0, L.W2_OFF + p0 + m),
                    hbin[b][:],
                    start=True,
                    stop=True,
                    **kwargs,
                )

            def finish_bank(k):
                rows = L.bank_rows[k]
                nc.scalar.activation(
                    y_t[k][:], bank_ps[k][:], sigmoid, bias=b2_ap(k, rows)
                )
                for b in L.banks[k]:
                    m = len(L.bins[b])
                    r0 = L.rowbase[b]
                    c0 = L.bin_off[b]
                    d = nc.sync.dma_start(
                        y_e[0:m, c0 : c0 + L.bin_w[b]],
                        y_t[k][r0 : r0 + m, 0 : L.bin_w[b]],
                    )
                    out_dma_names.append(d.ins.name)

            def finish_slot(j):
                b = L.slot_bin[j]
                bin_left[b] -= 1
                if bin_left[b] == 0:
                    mm2_bin(b)
                    k = L.bank_of[b]
                    bank_left[k] -= 1
                    if bank_left[k] == 0:
                        finish_bank(k)

            # software-pipelined emission: mm1 of pair p+1 runs on PE while
            # DVE does relu of pair p; bin mm2s/sigmoids fire as bins fill.
            stage = []  # (j, p1)
            for p in range(P + 1):
                if p < P:
                    nxt = [(j, mm1(j)) for j in L.pairs[p]]
                else:
                    nxt = []
                for j, p1 in stage:
                    relu(j, p1)
                    finish_slot(j)
                stage = nxt

            assert all(v == 0 for v in bank_left), "unemitted bank"

    _filter_drain_waits(nc, out_dma_names)
    _split_multi_waits(nc)
    return nc


# ---------------------------------------------------------------------------
# Entry point.
# ---------------------------------------------------------------------------
def _run(inputs, trace=False):
    x = np.asarray(inputs["x"], dtype=np.float32)
    num = np.asarray(inputs["num"])
    c = np.asarray(inputs["c"])
    W1 = np.asarray(inputs["W1"], dtype=np.float32)
    b1 = np.asarray(inputs["b1"], dtype=np.float32)
    W2 = np.asarray(inputs["W2"], dtype=np.float32)
    b2 = np.asarray(inputs["b2"], dtype=np.float32)

    B = x.shape[0]
    e = c[num].astype(np.int64)
    widths, slots = _plan(e)
    L = _Layout(widths)
    S = L.S

    x_bf = x.astype(NP_BF16)
    W1_bf = W1.astype(NP_BF16)
    W2_bf = W2.astype(NP_BF16)

    in_maps = []
    for core in range(N_CORES):
        data_c = np.zeros((128, L.DCOLS), dtype=NP_BF16)
        b1_c = np.zeros((128, S), dtype=np.float32)
        b2_c = np.zeros((128, L.NBANK), dtype=np.float32)
        for j in range(S):
            ex, idx = slots[core][j]
            r0 = 64 * L.hi_of[j]
            w0, w1c = L.w1_cols(j)
            c0, _ = L.xt_cols(j)
            if len(idx):
                data_c[r0 : r0 + 64, c0 : c0 + len(idx)] = x_bf[idx].T
            data_c[r0 : r0 + 64, w0:w1c] = W1_bf[ex]
            data_c[:, L.W2_OFF + L.slot_pos[j]] = W2_bf[ex, :, 0]
            b1_c[:, j] = b1[ex]
            bj = L.slot_bin[j]
            b2_c[L.rowbase[bj] + L.slot_row[j], L.bank_of[bj]] = b2[ex, 0]
        data_c[:, : 2 * S] = b1_c.view(NP_BF16)
        data_c[:, 2 * S : 2 * S + 2 * L.NBANK] = b2_c.view(NP_BF16)
        in_maps.append({"data": data_c})

    nc = _build(L)
    res = run_bass_kernel_spmd(nc, in_maps, list(range(N_CORES)), trace=trace)

    out = np.empty((B, 1), dtype=np.float32)
    for core in range(N_CORES):
        y_c = res.results[core]["y"]
        for j in range(S):
            ex, idx = slots[core][j]
            if len(idx):
                out[idx, 0] = y_c[
                    L.slot_row[j], L.slot_y_off[j] : L.slot_y_off[j] + len(idx)
                ]
    return out, res


def kernel(**inputs) -> np.ndarray:
    out, _ = _run(inputs, trace=False)
    return out


# revision 63
# speedup vs baseline: 1.0976x; 1.0192x over previous
"""MoE routing kernel for Trainium2 (8 NeuronCores).

Reference computation (B=16384, IN=64, HID=128, OUT=1, E=64, NMAP=1000):
    e = c[num]                                  # [B] expert id per sample
    h = relu(x @ W1[e] + b1[e])                 # [B, HID]
    y = sigmoid(h @ W2[e] + b2[e])              # [B, OUT]

Strategy: sort-by-expert dispatch on the host (the routing is pure
integer bookkeeping), dense per-expert matmuls on device. Each core gets
the same static slot structure (SPMD: one graph for all 8 cores); slot
widths are equalized across cores by snake-dealing the per-expert chunks
in descending size order, padding each slot to the max width over cores.

Device layout (per core): slots are paired onto the 128 SBUF partitions
— pair p puts slot 2p's x^T on partitions 0:64 and slot 2p+1's on
64:128. Full-width DMA, and the two K=64 matmuls of a pair run
concurrently in disjoint PE row groups. Slots are also first-fit packed
into "bins" of <=512 y columns: each bin is one PSUM bank, one
block-diagonal mm2 (lhsT = the bin's w2 columns), and one sigmoid.
All tensor data is bf16 (rel-err budget 2e-2); accumulation stays f32.

Per slot j (width Wj <= 512, pair p, bin b):
    mm1:   psum1[HID=128, Wj] = W1_j[64,128].T @ xT[64, Wj]      (PE)
    relu:  hbin_b[:, cj:cj+Wj] = bf16(max(psum1 + b1_j, 0))      (DVE)
Per bin b (M slots, width Wb <= 512):
    mm2:   psum2[M, Wb] = w2_bin[128,M].T @ hbin_b[128, Wb]      (PE)
    sig:   y[0:M, bin] = sigmoid(psum2 + b2_bin[M,1])            (ACT)
Slot j's outputs live in y[row_of_j_in_bin, its columns] (the
off-diagonal rows are garbage the host ignores).
"""

import os
import sys

if "/opt/trn_rl_repo" not in sys.path:
    sys.path.insert(0, "/opt/trn_rl_repo")

import numpy as np

import concourse.bass as bass
import concourse.mybir as mybir
from concourse import tile
from concourse.bass_utils import run_bass_kernel_spmd

N_CORES = 8
IN = 64
HID = 128
E = 64
MAX_W = 512  # moving-operand / PSUM-bank limit

BF16 = mybir.dt.bfloat16
F32 = mybir.dt.float32
NP_BF16 = mybir.dt.np(BF16)


# ---------------------------------------------------------------------------
# This container's walrus build rejects more than one sync wait per
# instruction ("Too many sync wait commands"). Post-pass over the lowered
# BIR: move the extra waits onto single-wait NOPs inserted just before the
# instruction on the same engine (program order makes this equivalent).
# ---------------------------------------------------------------------------
def _split_multi_waits(nc):
    ctr = 0
    for f in nc.m.functions:
        for blk in f.blocks:
            new_list = []
            for ins in blk.instructions:
                si = ins.sync_info
                if si is not None and si.on_wait and len(si.on_wait) > 1:
                    waits = list(si.on_wait)
                    for w in waits[:-1]:
                        ctr += 1
                        new_list.append(
                            mybir.InstNoOp(
                                name=f"waitsplit-{ctr}",
                                engine=ins.engine,
                                bass_nofuse=True,
                                sync_info=mybir.SyncInfo(
                                    on_wait=[w], on_update=[]
                                ),
                            )
                        )
                    si.on_wait = waits[-1:]
                    ins.sync_info = si
                new_list.append(ins)
            blk.instructions = new_list


def _filter_drain_waits(nc, out_dma_names):
    """The kernel-tail drain only needs to gate on the output DMAs'
    completion semaphores — every other wait Tile put on it is
    transitively implied. Fewer waits = fewer single-wait NOPs."""
    keep_ids = set()
    drain = None
    for f in nc.m.functions:
        for blk in f.blocks:
            for ins in blk.instructions:
                if ins.name in out_dma_names and ins.sync_info is not None:
                    for u in ins.sync_info.on_update:
                        keep_ids.add(u.id)
                if isinstance(ins, mybir.InstDrain):
                    si = ins.sync_info
                    if si is not None and len(si.on_wait) > 1:
                        drain = ins
    if drain is None or not keep_ids:
        return
    si = drain.sync_info
    kept = [w for w in si.on_wait if w.id in keep_ids]
    if kept:
        si.on_wait = kept
        drain.sync_info = si


def _slim_drain_and_barrier(self, tick_clock, wait_clock):
    """Replacement for TileContext._drain_and_barrier: the NEFF here runs
    exactly once per load (run_bass_via_pjrt → single execute), so skip
    the semaphore re-zeroing and the end barriers entirely."""
    drain_inst = self.nc.sync.drain()
    wait_clock.add_sem_waits(
        drain_inst.ins, tile.ScopedClock({None: tick_clock.global_clock})
    )
    popped = self.nc._tile_sem_poison_stack.pop()
    assert popped is self._sem_poison


tile.TileContext._drain_and_barrier = _slim_drain_and_barrier


# ---------------------------------------------------------------------------
# Host-side routing: build the per-core slot structure.
# ---------------------------------------------------------------------------
def _plan(e: np.ndarray):
    """Return (slot_widths, per_core_slots) where per_core_slots[i] is a list
    of (expert_id, sample_indices) aligned with slot_widths (desc order)."""
    order = np.argsort(e, kind="stable")
    counts = np.bincount(e, minlength=max(E, int(e.max()) + 1 if len(e) else E))
    starts = np.concatenate([[0], np.cumsum(counts)])

    chunks = []  # (width, expert, indices)
    for ex in range(len(counts)):
        idx = order[starts[ex] : starts[ex + 1]]
        for pos in range(0, len(idx), MAX_W):
            sub = idx[pos : pos + MAX_W]
            chunks.append((len(sub), ex, sub))
    chunks.sort(key=lambda t: -t[0])

    per_core = [[] for _ in range(N_CORES)]
    for r in range(0, len(chunks), N_CORES):
        row = chunks[r : r + N_CORES]
        cores = range(N_CORES) if (r // N_CORES) % 2 == 0 else range(N_CORES - 1, -1, -1)
        for ch, core in zip(row, cores):
            per_core[core].append(ch)

    n_slots = max(len(s) for s in per_core)
    empty = np.zeros((0,), dtype=np.int64)
    for s in per_core:
        while len(s) < n_slots:
            s.append((0, 0, empty))
        s.sort(key=lambda t: -t[0])

    widths = [max(per_core[i][j][0] for i in range(N_CORES)) for j in range(n_slots)]
    widths = [max(w, 1) for w in widths]
    slots = [[(s[j][1], s[j][2]) for j in range(n_slots)] for s in per_core]
    return widths, slots


class _Layout:
    """Column layout shared by the graph builder and the host packer.

    data tensor (bf16 cols):
      [0, 2S)          b1 columns, f32 bitcast (col j = b1 of slot j)
      [2S, 2S+2NB)     b2 columns, f32 bitcast (col b, partition i = b2 of
                       bins[b][i])
      [HDR, ...)       per pair p: W1_p (HID cols, slot 2p on partitions
                       0:64, slot 2p+1 on 64:128) then xT_p (pw_p cols,
                       same stacking); pairs 0, 1, 2.. in order
      [W2_OFF, +S)     w2 columns in bin order (col slot_pos[j])
    Input DMA split: A = header + w2 + pair0 (sync), B = pair1 (scalar),
    C = pairs 2.. (sync).
    """

    def __init__(self, widths):
        S = len(widths)
        P = (S + 1) // 2
        self.widths = widths
        self.S, self.P = S, P
        self.NT = int(np.sum(widths))

        bins, bin_w = [], []
        self.slot_bin = [0] * S
        for j in range(S):
            for b in range(len(bins)):
                if bin_w[b] + widths[j] <= MAX_W:
                    bins[b].append(j)
                    bin_w[b] += widths[j]
                    self.slot_bin[j] = b
                    break
            else:
                self.slot_bin[j] = len(bins)
                bins.append([j])
                bin_w.append(widths[j])
        # the last bin completes last: keep its final slot solo so the
        # endgame mm2+sigmoid chain is as short as possible
        if (
            os.environ.get("K_SOLOBIN", "0") == "1"
            and len(bins[-1]) > 1
            and len(bins) < 7
        ):
            j = bins[-1].pop()
            bin_w[-1] -= widths[j]
            self.slot_bin[j] = len(bins)
            bins.append([j])
            bin_w.append(widths[j])
        self.bins, self.bin_w = bins, bin_w
        self.NB = len(bins)
        self.Mmax = max(len(bs) for bs in bins)

        # banks: two bins share one PSUM bank — the second bin's mm2
        # lands at partition 32 (PE col-group tiling), so the paired
        # mm2s overlap on the PE and one sigmoid covers both bins.
        # The last bin stays solo so the final y DMA stays small.
        self.bank_of = [0] * self.NB
        self.rowbase = [0] * self.NB
        self.banks = []  # list of [bin ids]
        b = 0
        while b < self.NB:
            if b + 2 < self.NB or (b + 2 == self.NB and self.NB % 2 == 0):
                pair = [b, b + 1]
            else:
                pair = [b]
            for i, bb in enumerate(pair):
                self.bank_of[bb] = len(self.banks)
                self.rowbase[bb] = 32 * i
            self.banks.append(pair)
            b += len(pair)
        self.NBANK = len(self.banks)
        self.bank_w = [
            max(bin_w[bb] for bb in pair) for pair in self.banks
        ]

        # slot processing order: bank by bank, round-robin across the
        # bank's bins so both bins complete close together and their
        # col-group mm2s issue back-to-back (overlapping on the PE)
        self.proc = []
        for pair in self.banks:
            rows = [list(bins[bb]) for bb in pair]
            while any(rows):
                for r in rows:
                    if r:
                        self.proc.append(r.pop(0))

        self.slot_y_off = [0] * S  # column in y / position of slot's range
        self.slot_row = [0] * S  # row in y
        self.slot_pos = [0] * S  # w2 column
        self.bin_off = []
        off = pos = 0
        for b, bs in enumerate(bins):
            self.bin_off.append(off)
            for i, j in enumerate(bs):
                self.slot_y_off[j] = off
                self.slot_row[j] = i
                self.slot_pos[j] = pos
                off += widths[j]
                pos += 1
        assert off == self.NT

        # pairs follow the processing order: pair k stacks proc[2k] on
        # partitions 0:64 and proc[2k+1] on 64:128
        self.pair_of = {}
        self.hi_of = {}
        self.pairs = []
        for k in range(P):
            js = self.proc[2 * k : 2 * k + 2]
            self.pairs.append(js)
            for hi, j in enumerate(js):
                self.pair_of[j] = k
                self.hi_of[j] = hi
        self.pw = [
            max(widths[j] for j in js) + (max(widths[j] for j in js) & 1)
            for js in self.pairs
        ]

        # per-bank psum/sbuf row extent (bins at partition 0 and 32)
        self.bank_rows = [
            self.rowbase[pair[-1]] + len(bins[pair[-1]]) for pair in self.banks
        ]

        self.HDR = 2 * S + 2 * self.NBANK
        self.pair_base = []
        c = self.HDR
        for k in range(P):
            self.pair_base.append(c)
            c += HID + self.pw[k]
        self.W2_OFF = c
        self.DCOLS = c + S + (S & 1)
        self.CUT1 = self.pair_base[1] if P > 1 else self.W2_OFF
        self.CUT2 = self.pair_base[2] if P > 2 else self.W2_OFF

    def w1_cols(self, j):
        p = self.pair_of[j]
        return self.pair_base[p], self.pair_base[p] + HID

    def xt_cols(self, j):
        p = self.pair_of[j]
        c0 = self.pair_base[p] + HID
        return c0, c0 + self.widths[j]


# ---------------------------------------------------------------------------
# Device graph builder (shared by all cores).
# ---------------------------------------------------------------------------
def _build(L: _Layout):
    S, P, NB = L.S, L.P, L.NB
    widths = L.widths

    nc = bass.Bass("TRN2", target_bir_lowering=False, debug=False)
    data_e = nc.declare_dram_parameter("data", [128, L.DCOLS], BF16, isOutput=False)
    y_e = nc.declare_dram_parameter("y", [L.Mmax, L.NT], F32, isOutput=True)

    sigmoid = mybir.ActivationFunctionType.Sigmoid
    add = mybir.AluOpType.add
    amax = mybir.AluOpType.max

    out_dma_names = []
    with tile.TileContext(nc) as tc:
        with (
            tc.tile_pool(name="sb", bufs=1) as sb,
            tc.tile_pool(
                name="ps1", bufs=max(1, min(5, 8 - L.NBANK)), space="PSUM"
            ) as ps1,
            tc.tile_pool(name="ps2", bufs=1, space="PSUM") as ps2,
            tc.tile_pool(name="dummy", bufs=1) as dummy_pool,
        ):
            # Engine preloads during the input DMA window (all on garbage
            # SBUF, no data deps): ACT sigmoid table load, DVE first-op
            # cost, PE pipeline priming. The warmup matmuls rotate through
            # the same psum bufs the real mm1s use (PE executes in order).
            WARMUP = os.environ.get("K_WARMUP", "1") == "1"
            if WARMUP:
                warm = dummy_pool.tile([128, 512], BF16)
                warm_in = dummy_pool.tile([1, 16], F32)
                warm_y = dummy_pool.tile([1, 16], F32)
                warm_v = dummy_pool.tile([1, 16], F32)
                nc.gpsimd.memset(warm[:], 0.0)
                nc.gpsimd.memset(warm_in[:], 0.0)
                nc.scalar.activation(warm_y[:], warm_in[:], sigmoid)
                nc.vector.tensor_scalar(
                    warm_v[:], warm_in[:], 0.0, 0.0, add, amax
                )
                for _ in range(int(os.environ.get("K_NWARM", "4"))):
                    warm_ps = ps1.tile([HID, 448], F32, tag="p1")
                    nc.tensor.matmul(
                        warm_ps[:], warm[:, :128], warm[:, :448],
                        start=True, stop=True,
                    )

            dataA = sb.tile([128, L.CUT1], BF16)
            dataB = sb.tile([128, max(L.CUT2 - L.CUT1, 1)], BF16)
            dataC = sb.tile([128, max(L.DCOLS - L.CUT2, 1)], BF16)
            y_t = []
            for k in range(L.NBANK):
                yt = sb.tile([L.bank_rows[k], L.bank_w[k]], F32, tag=f"y{k}")
                y_t.append(yt)
            hbin = []
            for b in range(NB):
                hb = sb.tile([HID, L.bin_w[b]], BF16, tag=f"h{b}")
                hbin.append(hb)

            nc.sync.dma_start(dataA[:], data_e[:, : L.CUT1])
            if L.CUT2 > L.CUT1:
                nc.scalar.dma_start(dataB[:], data_e[:, L.CUT1 : L.CUT2])
            if L.DCOLS > L.CUT2:
                nc.sync.dma_start(dataC[:], data_e[:, L.CUT2 :])

            def dcols(c0, c1, r0=0, r1=128):
                if c1 <= L.CUT1:
                    return dataA[r0:r1, c0:c1]
                if c1 <= L.CUT2:
                    assert c0 >= L.CUT1
                    return dataB[r0:r1, c0 - L.CUT1 : c1 - L.CUT1]
                assert c0 >= L.CUT2
                return dataC[r0:r1, c0 - L.CUT2 : c1 - L.CUT2]

            def b1_ap(j):
                return dataA[:, 2 * j : 2 * j + 2].bitcast(F32)

            def b2_ap(k, rows):
                c = 2 * S + 2 * k
                return dataA[0:rows, c : c + 2].bitcast(F32)

            def mm1(j):
                r0 = 64 * L.hi_of[j]
                c0, c1 = L.xt_cols(j)
                w0, w1c = L.w1_cols(j)
                p1 = ps1.tile([HID, widths[j]], F32, tag="p1")
                nc.tensor.matmul(
                    p1[:],
                    dcols(w0, w1c, r0, r0 + 64),
                    dcols(c0, c1, r0, r0 + 64),
                    start=True,
                    stop=True,
                )
                return p1

            n_act_relu = int(os.environ.get("K_ACTRELU", "1"))
            act_relu = set(L.proc[:n_act_relu])
            relu_fn = mybir.ActivationFunctionType.Relu

            def relu(j, p1):
                b = L.slot_bin[j]
                c0 = L.slot_y_off[j] - L.bin_off[b]
                out = hbin[b][:, c0 : c0 + widths[j]]
                if j in act_relu:
                    # ACT is idle until the first sigmoid; offload the
                    # leading relu(s) there to unblock DVE earlier
                    nc.scalar.activation(out, p1[:], relu_fn, bias=b1_ap(j))
                else:
                    nc.vector.tensor_scalar(
                        out, p1[:], b1_ap(j), 0.0, add, amax
                    )

            bank_ps = []
            for k in range(L.NBANK):
                bp = ps2.tile([L.bank_rows[k], L.bank_w[k]], F32, tag=f"bk{k}")
                bank_ps.append(bp)
            bin_left = [len(bs) for bs in L.bins]
            bank_left = [len(pair) for pair in L.banks]

            def mm2_bin(b):
                m = len(L.bins[b])
                k = L.bank_of[b]
                r0 = L.rowbase[b]
                p0 = L.slot_pos[L.bins[b][0]]
                kwargs = {}
                if r0:
                    kwargs["tile_position"] = (0, r0)
                nc.tensor.matmul(
                    bank_ps[k][r0 : r0 + m, 0 : L.bin_w[b]],
                    dcols(L.W2_OFF + p0, L.W2_OFF + p0 + m),
                    hbin[b][:],
                    start=True,
                    stop=True,
                    **kwargs,
                )

            def finish_bank(k):
                rows = L.bank_rows[k]
                nc.scalar.activation(
                    y_t[k][:], bank_ps[k][:], sigmoid, bias=b2_ap(k, rows)
                )
                # non-final banks leave via the otherwise-idle GpSimd
                # (SWDGE) so the Sync queue is free to issue the final
                # bank's DMA the moment its sigmoid lands
                eng = nc.sync if k == L.NBANK - 1 else nc.gpsimd
                for b in L.banks[k]:
                    m = len(L.bins[b])
                    r0 = L.rowbase[b]
                    c0 = L.bin_off[b]
                    d = eng.dma_start(
                        y_e[0:m, c0 : c0 + L.bin_w[b]],
                        y_t[k][r0 : r0 + m, 0 : L.bin_w[b]],
                    )
                    out_dma_names.append(d.ins.name)

            def finish_slot(j):
                b = L.slot_bin[j]
                bin_left[b] -= 1
                if bin_left[b] == 0:
                    mm2_bin(b)
                    k = L.bank_of[b]
                    bank_left[k] -= 1
                    if bank_left[k] == 0:
                        finish_bank(k)

            # software-pipelined emission: mm1 of pair p+1 runs on PE while
            # DVE does relu of pair p; bin mm2s/sigmoids fire as bins fill.
            stage = []  # (j, p1)
            for p in range(P + 1):
                if p < P:
                    nxt = [(j, mm1(j)) for j in L.pairs[p]]
                else:
                    nxt = []
                for j, p1 in stage:
                    relu(j, p1)
                    finish_slot(j)
                stage = nxt

            assert all(v == 0 for v in bank_left), "unemitted bank"

    _filter_drain_waits(nc, out_dma_names)
    _split_multi_waits(nc)
    return nc


# ---------------------------------------------------------------------------
# Entry point.
# ---------------------------------------------------------------------------
def _run(inputs, trace=False):
    x = np.asarray(inputs["x"], dtype=np.float32)
    num = np.asarray(inputs["num"])
    c = np.asarray(inputs["c"])
    W1 = np.asarray(inputs["W1"], dtype=np.float32)
    b1 = np.asarray(inputs["b1"], dtype=np.float32)
    W2 = np.asarray(inputs["W2"], dtype=np.float32)
    b2 = np.asarray(inputs["b2"], dtype=np.float32)

    B = x.shape[0]
    e = c[num].astype(np.int64)
    widths, slots = _plan(e)
    L = _Layout(widths)
    S = L.S

    x_bf = x.astype(NP_BF16)
    W1_bf = W1.astype(NP_BF16)
    W2_bf = W2.astype(NP_BF16)

    in_maps = []
    for core in range(N_CORES):
        data_c = np.zeros((128, L.DCOLS), dtype=NP_BF16)
        b1_c = np.zeros((128, S), dtype=np.float32)
        b2_c = np.zeros((128, L.NBANK), dtype=np.float32)
        for j in range(S):
            ex, idx = slots[core][j]
            r0 = 64 * L.hi_of[j]
            w0, w1c = L.w1_cols(j)
            c0, _ = L.xt_cols(j)
            if len(idx):
                data_c[r0 : r0 + 64, c0 : c0 + len(idx)] = x_bf[idx].T
            data_c[r0 : r0 + 64, w0:w1c] = W1_bf[ex]
            data_c[:, L.W2_OFF + L.slot_pos[j]] = W2_bf[ex, :, 0]
            b1_c[:, j] = b1[ex]
            bj = L.slot_bin[j]
            b2_c[L.rowbase[bj] + L.slot_row[j], L.bank_of[bj]] = b2[ex, 0]
        data_c[:, : 2 * S] = b1_c.view(NP_BF16)
        data_c[:, 2 * S : 2 * S + 2 * L.NBANK] = b2_c.view(NP_BF16)
        in_maps.append({"data": data_c})

    nc = _build(L)
    res = run_bass_kernel_spmd(nc, in_maps, list(range(N_CORES)), trace=trace)

    out = np.empty((B, 1), dtype=np.float32)
    for core in range(N_CORES):
        y_c = res.results[core]["y"]
        for j in range(S):
            ex, idx = slots[core][j]
            if len(idx):
                out[idx, 0] = y_c[
                    L.slot_row[j], L.slot_y_off[j] : L.slot_y_off[j] + len(idx)
                ]
    return out, res


def kernel(**inputs) -> np.ndarray:
    out, _ = _run(inputs, trace=False)
    return out
